# revision 34
# baseline (speedup 1.0000x reference)
"""Trainium2 Bass kernel for nn_DiscreteLoss (data-parallel over batch).

Contract: kernel(**inputs) takes the FULL unsharded inputs (B=64) and
returns the FULL scalar loss.  Internally the batch dim is sharded over
8 NeuronCores (8 batches each); each core produces per-partition partial
sums for every loss term, which the host combines in float64.

Device strategy per core (fp8 everywhere the math allows):
  - All bulk tensors ship as TRN fp8e4 (E4M3), halving HBM traffic vs
    bf16 (~18.8KB/partition total).  Host pre-scales fold every
    normalizer into one of two shared divisors: rz/zs x4, masks /16,
    MARK'ed pts cols x sqrt(2) (folds the landmark term into the disk
    sum), and the "best" pack columns are pre-scaled so one accumulator
    column IS the epilogue term (marked best_pt cols carry the
    sqrt(1+B*N*2) landmark weight; best_mask cols outside [32:96) are
    zeroed to express the slice).
  - The mapping gather AND the ground-truth subtraction run as ONE
    DoubleRow fp8 matmul per 512-col block: lhsT k-subtile 0 = one-hot
    E_b (host-built, exact in fp8), k-subtile 1 = -I (stepped-slice AP
    pairs slot b with slot 8), rhs k0 = values, k1 = gt.  PSUM receives
    (gathered - gt) exactly (all products are 1.0*x or -1.0*x).
  - Squares+reduction of the PSUM diffs: ACT runs activation(Square,
    accum_out); DVE cannot dual-read PSUM, so its share runs as
    bn_stats per 512-block and the host reconstructs sum(x^2) =
    M2 + n*mean^2 from the tiny stats.  Pool has no PSUM port and
    walrus rejects tensor ops on it, so ACT+DVE split all of it.
  - KLD uses Ln(qy * V): the activation scale folds log(V) in, so one
    DVE multiply-accum gives sum qy*(log qy - log(1/V)); no separate
    sum(qy) pass.  Ln is the first activation so Square reuses its
    (natural_log) table -> exactly one ACT table load.
  - A burst of dummy matmuls at t=0 ramps the PE out of its low p-state
    before the real matmuls arrive.
  - DMA layout respects two measured HWDGE properties: the ring keeps
    only ~2 DMAs in flight, and each completion semaphore needs ~2us to
    collect its 16 per-engine increments -- so few chunks, each >=
    ~4KB/partition, a small lead chunk, and a small tail chunk; output
    partials store in two DMAs (early columns / tail columns).
"""

import contextlib
import ctypes
import os
import sys
import types

for _p in ("/opt/trn_rl_repo", "/root/.axon_site/_ro/trn_rl_repo"):
    if os.path.isdir(_p) and _p not in sys.path:
        sys.path.append(_p)

import numpy as np

# --- problem constants (hardcoded per spec) ---
B, S, N, D, V = 64, 128, 128, 512, 128
N_CORES = 8
BPC = B // N_CORES          # batches per core = 8
ALPHA, BETA, GAMMA, EPS = 1.0, 0.1, 1.0, 1e-20
MARK = (0, 29, 88, 117)

N_WARMUP = 36               # dummy matmuls to ramp the PE p-state
BAT_BYTES = 2048            # fp8 bytes per batch per partition

_CACHE = {}


def _install_ntff_hook_shim():
    """run_bass_kernel_spmd(trace=True) looks for antenv.axon_hooks, which
    this image lacks; recreate the ctypes hook against libaxon_pjrt.so."""
    if "antenv.axon_hooks" in sys.modules:
        return
    so_path = "/opt/axon/libaxon_pjrt.so"

    def _get_hook():
        if not os.path.exists(so_path):
            return None
        lib = ctypes.CDLL(so_path)
        if not hasattr(lib, "axon_start_nrt_profile"):
            return None
        lib.axon_start_nrt_profile.argtypes = [
            ctypes.POINTER(ctypes.c_int64), ctypes.c_size_t]
        lib.axon_start_nrt_profile.restype = ctypes.c_int64
        lib.axon_stop_nrt_profile.argtypes = [ctypes.c_char_p]
        lib.axon_stop_nrt_profile.restype = ctypes.c_int64

        @contextlib.contextmanager
        def _hook(output_dir, device_ids):
            import jax
            jax.devices()
            if device_ids:
                ids = (ctypes.c_int64 * len(device_ids))(*device_ids)
                rc = lib.axon_start_nrt_profile(ids, len(device_ids))
            else:
                rc = lib.axon_start_nrt_profile(None, 0)
            if rc != 0:
                raise RuntimeError(f"axon_start_nrt_profile rc={rc}")
            try:
                yield
            finally:
                n = lib.axon_stop_nrt_profile(str(output_dir).encode())
                if n < 0:
                    raise RuntimeError(f"axon_stop_nrt_profile rc={n}")

        return _hook

    mod = types.ModuleType("antenv.axon_hooks")
    mod.get_axon_ntff_profile_hook = _get_hook
    mod.set_axon_ntff_profile_hook = lambda h: None
    sys.modules["antenv.axon_hooks"] = mod


def _build_program():
    import concourse.bacc as bacc
    import concourse.tile as tile
    from concourse import mybir

    f32 = mybir.dt.float32
    bf16 = mybir.dt.bfloat16
    fp8 = mybir.dt.float8e4
    nc = bacc.Bacc(None, target_bir_lowering=False, debug=False)

    SQUARE = mybir.ActivationFunctionType.Square
    LN = mybir.ActivationFunctionType.Ln
    AL = mybir.AluOpType
    DR = mybir.MatmulPerfMode.DoubleRow

    # ---- per-core DRAM parameters (host pre-packed) ----
    # e2: one-hot gather pack, e[j, b*128+s] = (mapping[b,s] == j); slot 8
    # holds -I.  qb: [0:1024] qy (qb[v, b*128+s] = qy[b,s,v]), [1024:1280]
    # the "best" pack as bf16 bytes (cols [0:64] = A-halves, [64:128] =
    # B-halves, pre-scaled so one accum col == the epilogue term).
    d_e2 = nc.declare_dram_parameter("e2", [128, 1152], fp8, isOutput=False)
    d_qb = nc.declare_dram_parameter("qb", [128, 1280], fp8, isOutput=False)
    # per batch 2048B: [rz*4 | zs*4 | pts.w | masks/16 | pts_gt.w | masks_gt/16]
    d_bat = nc.declare_dram_parameter("bat", [128, BPC * BAT_BYTES], fp8, isOutput=False)
    d_out = nc.declare_dram_parameter("o_part", [128, 64], f32, isOutput=True)

    with tile.TileContext(nc) as tc:
        with contextlib.ExitStack() as ctx:
            sb = ctx.enter_context(tc.tile_pool(name="sb", bufs=1))
            psp = ctx.enter_context(tc.tile_pool(name="ps", bufs=1, space="PSUM"))

            # ---- tiles ----
            t_e2 = sb.tile([128, 1152], fp8)
            t_qb = sb.tile([128, 1280], fp8)
            t_bat = sb.tile([128, BPC * BAT_BYTES], fp8)
            t_ones = sb.tile([1, 128], bf16)
            t_lnq = sb.tile([128, BPC * V], bf16)
            t_jk = sb.tile([128, BPC * V], bf16)      # kld junk out
            t_ja = sb.tile([128, 2048], bf16)         # ACT square junk out
            t_ed = sb.tile([128, 64], f32)            # epilogue diffs
            t_ej = sb.tile([128, 64], f32)            # epilogue junk out
            a_out = sb.tile([128, 64], f32)
            ps = psp.tile([128, 4096], f32)           # all 8 PSUM banks

            e3 = t_e2[:].rearrange("p (a m) -> p a m", m=128)
            t_qy = t_qb[:, 0:1024]
            t_best = t_qb[:, 1024:1280].bitcast(bf16)     # [128, 128]

            # ---- PE warm-up: dummy matmuls ramp the p-state ----
            nc.vector.memset(t_ones[:], 1.0)
            for _ in range(N_WARMUP):
                nc.tensor.matmul(ps[:, 3072:3200], lhsT=t_ones[:],
                                 rhs=t_ones[:], start=True, stop=True)

            # ---- input DMAs (sync HWDGE ring, ordered by need) ----
            # The HWDGE ring keeps only ~2 DMAs in flight, and each DMA's
            # completion semaphore takes ~2us to collect its 16 per-engine
            # increments -- so use FEW chunks, each carrying >= ~2us of data
            # (>=4KB/partition), with a small lead chunk (b0) to start
            # compute early.
            def bat_dma(b_lo, b_hi):
                nc.sync.dma_start(
                    out=t_bat[:, b_lo * BAT_BYTES:b_hi * BAT_BYTES],
                    in_=d_bat.ap()[:, b_lo * BAT_BYTES:b_hi * BAT_BYTES])

            # header DMAs ride the ACT HWDGE ring (ACT is idle early) so
            # the sync ring starts streaming batch data immediately.
            nc.scalar.dma_start(out=t_e2[:], in_=d_e2.ap())
            nc.scalar.dma_start(out=t_qb[:], in_=d_qb.ap())
            bat_dma(0, 1)
            bat_dma(1, 3)
            bat_dma(3, 6)
            bat_dma(6, 8)

            # ---- KLD: Ln(qy*V) folds log(V); DVE does multiply+accum ----
            # fp8 qy is always >= 2^-10 (inputs are uniform(1e-3, 1)), so no
            # eps bias is needed under the log.  (Pool rejects
            # TensorScalarPtr; DVE dual-SBUF reads are fine — only dual-PSUM
            # reads are forbidden.)
            nc.scalar.activation(out=t_lnq[:], in_=t_qy[:], func=LN,
                                 scale=float(V), bias=0.0)
            nc.vector.scalar_tensor_tensor(
                out=t_jk[:], in0=t_qy[:], scalar=1.0, in1=t_lnq[:],
                op0=AL.bypass, op1=AL.mult, accum_out=a_out[:, 8:9])

            # ---- epilogue ("best" terms): DVE sub, DVE square-accum ----
            nc.vector.scalar_tensor_tensor(
                out=t_ed[:], in0=t_best[:, 0:64], scalar=0.0,
                in1=t_best[:, 64:128], op0=AL.bypass, op1=AL.subtract)
            nc.vector.scalar_tensor_tensor(
                out=t_ej[:], in0=t_ed[:], scalar=1.0, in1=t_ed[:],
                op0=AL.bypass, op1=AL.mult, accum_out=a_out[:, 9:10])

            # ---- fused gather-minus-gt matmuls (DoubleRow fp8) ----
            # batch b -> PSUM slot (b % 4) * 1024
            def mm_batch(b):
                lhsT = e3[:, b:9:8 - b, :]          # [128, 2, 128]: E_b, -I
                base = b * BAT_BYTES
                r = b % 4
                rhs1 = t_bat[:, base:base + 1024].rearrange(
                    "p (k c) -> p k c", k=2)
                rhs2 = t_bat[:, base + 1024:base + 2048].rearrange(
                    "p (k c) -> p k c", k=2)
                nc.tensor.matmul(ps[:, r * 1024:r * 1024 + 512], lhsT=lhsT,
                                 rhs=rhs1, start=True, stop=True, perf_mode=DR)
                nc.tensor.matmul(ps[:, r * 1024 + 512:r * 1024 + 1024],
                                 lhsT=lhsT, rhs=rhs2, start=True, stop=True,
                                 perf_mode=DR)

            def sq_act(lo_col, n_col, out_col):
                nc.scalar.activation(
                    out=t_ja[:, 0:n_col],
                    in_=ps[:, lo_col:lo_col + n_col], func=SQUARE,
                    accum_out=a_out[:, out_col:out_col + 1])

            def sq_dve(lo_col, n_col, out_col):
                # DVE may read only ONE operand from PSUM, so squares run as
                # bn_stats: per 512-block (count, mean, n*var) for even/odd
                # element sets; host reconstructs sum(x^2) = M2 + n*mean^2.
                for k in range(n_col // 512):
                    nc.vector.bn_stats(
                        out=a_out[:, out_col + 6 * k:out_col + 6 * k + 6],
                        in_=ps[:, lo_col + 512 * k:lo_col + 512 * (k + 1)])

            # a_out columns ordered by completion time so the early block
            # [0:28] can store while the tail [28:56] still computes.
            # Wave 1 uses only SINGLE-batch squares so each PSUM slot frees
            # as early as possible -- wave-2 matmuls are gated by wave-1
            # READS, not by data arrival.
            # b0: ACT single square (earliest possible engine start)
            mm_batch(0)
            sq_act(0, 1024, 0)
            # b1: ACT single; b2: DVE bn_stats (slots free ~1.5us sooner
            # than one ACT pair over [1024:3072])
            mm_batch(1)
            sq_act(1024, 1024, 1)
            mm_batch(2)
            sq_dve(2048, 1024, 16)
            # b3: DVE bn_stats
            mm_batch(3)
            sq_dve(3072, 1024, 32)
            # (b4,b5): ACT pair (slots 0,1 freed by sq0/sq1 reads)
            mm_batch(4); mm_batch(5)
            sq_act(0, 2048, 28)
            # b6, b7 (last chunk): halves split across both engines
            mm_batch(6)
            sq_act(2048, 512, 29)
            sq_dve(2560, 512, 44)
            mm_batch(7)
            sq_act(3072, 512, 30)
            sq_dve(3584, 512, 50)

            # ---- store partials: early block, then the tail block ----
            nc.sync.dma_start(out=d_out.ap()[:, 0:28], in_=a_out[:, 0:28])
            nc.sync.dma_start(out=d_out.ap()[:, 28:56], in_=a_out[:, 28:56])

    nc.compile()
    return nc


def _get_program():
    if "nc" not in _CACHE:
        _CACHE["nc"] = _build_program()
    return _CACHE["nc"]


def _shard_inputs(inputs):
    """Split the full B=64 inputs into 8 per-core input maps (fp8 packs)."""
    import ml_dtypes
    fp8 = ml_dtypes.float8_e4m3fn
    bf16 = ml_dtypes.bfloat16
    f = lambda k: np.asarray(inputs[k], dtype=np.float32)

    # pts columns: weight sqrt(2) on MARK'ed landmark points folds the
    # landmark sum into the disk sum (both /(B*S) in the reference).
    wpts = np.ones((N, 2), dtype=np.float32)
    wpts[list(MARK), :] = np.sqrt(np.float32(2.0))
    wpts_f = wpts.reshape(1, 1, N, 2)

    rz4 = f("rzs") * np.float32(4.0)                    # [B,S,D]
    zs4 = f("zs") * np.float32(4.0)
    ptsw = f("pts") * wpts_f                            # [B,S,N,2]
    ptsgw = f("pts_gt") * wpts_f
    msc = np.float32(1.0 / 16.0)
    mk = f("masks") * msc
    mkg = f("masks_gt") * msc

    # bat[j, b, 0:512]=rz4, [512:1024]=zs4, [1024:1280]=ptsw,
    # [1280:1536]=masks/16, [1536:1792]=pts_gt.w, [1792:2048]=masks_gt/16
    bat = np.empty((B, S, BAT_BYTES), dtype=np.float32)
    bat[:, :, 0:512] = rz4
    bat[:, :, 512:1024] = zs4
    bat[:, :, 1024:1280] = ptsw.reshape(B, S, 256)
    bat[:, :, 1280:1536] = mk.reshape(B, S, 256)
    bat[:, :, 1536:1792] = ptsgw.reshape(B, S, 256)
    bat[:, :, 1792:2048] = mkg.reshape(B, S, 256)
    bat8 = bat.astype(fp8)                              # [B, S, 2048]

    qy8 = f("qy").astype(fp8)                           # [B, S, V]

    # "best" pack scales: one accumulator column == the exact epilogue
    # loss contribution gamma*(best_reg + best_auto + alpha*best_seg).
    s_rz = np.float32(1.0 / np.sqrt(B * D))
    bn2 = B * N * 2
    s_pt = np.float32(1.0 / np.sqrt(float(bn2) * float(B * N)))
    w_pt = np.full((N, 2), s_pt, dtype=np.float32)
    w_pt[list(MARK), :] *= np.float32(np.sqrt(1.0 + bn2))
    w_mask = np.zeros((N, 2), dtype=np.float32)
    w_mask[32:96, :] = np.float32(1.0 / np.sqrt(B * 64 * 2))

    mapping = np.asarray(inputs["mapping"]).astype(np.int64)

    in_maps = []
    for c in range(N_CORES):
        lo, hi = c * BPC, (c + 1) * BPC
        m_core = mapping[lo:hi]                          # [8, 128]

        e = np.zeros((128, 9, 128), dtype=np.float32)
        bi = np.repeat(np.arange(BPC), S)
        si = np.tile(np.arange(S), BPC)
        e[m_core.reshape(-1), bi, si] = 1.0
        e[:, 8, :] = -np.eye(128, dtype=np.float32)

        # best pack halves [128, 64] each
        a_half = np.concatenate([
            (f("best_rz")[lo:hi] * s_rz).reshape(128, 32),
            (f("best_pt")[lo:hi] * w_pt).reshape(128, 16),
            (f("best_mask")[lo:hi] * w_mask).reshape(128, 16)], axis=1)
        b_half = np.concatenate([
            (f("logits")[lo:hi] * s_rz).reshape(128, 32),
            (f("best_pt_gt")[lo:hi] * w_pt).reshape(128, 16),
            (f("best_mask_gt")[lo:hi] * w_mask).reshape(128, 16)], axis=1)
        best = np.concatenate([a_half, b_half], axis=1).astype(bf16)

        qb = np.empty((128, 1280), dtype=fp8)
        qb[:, 0:1024] = qy8[lo:hi].transpose(2, 0, 1).reshape(128, BPC * V)
        qb[:, 1024:1280] = np.ascontiguousarray(best).view(np.uint8).view(fp8)

        m = {
            "e2": e.reshape(128, 9 * 128).astype(fp8),
            "qb": qb,
            "bat": np.ascontiguousarray(
                bat8[lo:hi].transpose(1, 0, 2).reshape(128, BPC * BAT_BYTES)),
        }
        in_maps.append(m)
    return in_maps


def _combine(results):
    """Host-side float64 reduction of the per-core partial sums."""

    def bn_sumsq(o, lo, blocks):
        # per 512-block bn_stats sextet: [cnt_e, mean_e, M2_e, cnt_o,
        # mean_o, M2_o]; sum(x^2) = M2 + cnt * mean^2 for each half.
        v = o[:, lo:lo + 6 * blocks].reshape(128, blocks, 2, 3)
        cnt, mean, m2 = v[..., 0], v[..., 1], v[..., 2]
        return (m2 + cnt * mean * mean).sum()

    s_main = s_kld = s_epi = 0.0
    for r in results:
        o = r["o_part"].astype(np.float64)
        s_main += (o[:, 0].sum() + o[:, 1].sum() + o[:, 28].sum()
                   + o[:, 29].sum() + o[:, 30].sum())
        s_main += (bn_sumsq(o, 16, 2) + bn_sumsq(o, 32, 2)
                   + bn_sumsq(o, 44, 1) + bn_sumsq(o, 50, 1))
        s_kld += o[:, 8].sum()
        s_epi += o[:, 9].sum()
    ret = s_main / (B * S) + BETA * s_kld / (B * S) + GAMMA * s_epi
    return np.float32(ret * B)


def run_sharded(inputs, trace=False):
    """Compile (cached), run on the 8 cores, return (scalar, BassKernelResults)."""
    _install_ntff_hook_shim()
    from concourse.bass_utils import run_bass_kernel_spmd

    nc = _get_program()
    in_maps = _shard_inputs(inputs)
    res = run_bass_kernel_spmd(nc, in_maps, list(range(N_CORES)), trace=trace)
    return _combine(res.results), res


def kernel(**inputs) -> np.ndarray:
    out, _ = run_sharded(inputs, trace=False)
    return out


# revision 35
# speedup vs baseline: 1.0476x; 1.0476x over previous
"""Trainium2 Bass kernel for nn_DiscreteLoss (data-parallel over batch).

Contract: kernel(**inputs) takes the FULL unsharded inputs (B=64) and
returns the FULL scalar loss.  Internally the batch dim is sharded over
8 NeuronCores (8 batches each); each core produces per-partition partial
sums for every loss term, which the host combines in float64.

Device strategy per core (fp8 everywhere the math allows):
  - All bulk tensors ship as TRN fp8e4 (E4M3), halving HBM traffic vs
    bf16 (~18.8KB/partition total).  Host pre-scales fold every
    normalizer into one of two shared divisors: rz/zs x4, masks /16,
    MARK'ed pts cols x sqrt(2) (folds the landmark term into the disk
    sum), and the "best" pack columns are pre-scaled so one accumulator
    column IS the epilogue term (marked best_pt cols carry the
    sqrt(1+B*N*2) landmark weight; best_mask cols outside [32:96) are
    zeroed to express the slice).
  - The mapping gather AND the ground-truth subtraction run as ONE
    DoubleRow fp8 matmul per 512-col block: lhsT k-subtile 0 = one-hot
    E_b (host-built, exact in fp8), k-subtile 1 = -I (stepped-slice AP
    pairs slot b with slot 8), rhs k0 = values, k1 = gt.  PSUM receives
    (gathered - gt) exactly (all products are 1.0*x or -1.0*x).
  - Squares+reduction of the PSUM diffs: ACT runs activation(Square,
    accum_out); DVE cannot dual-read PSUM, so its share runs as
    bn_stats per 512-block and the host reconstructs sum(x^2) =
    M2 + n*mean^2 from the tiny stats.  Pool has no PSUM port and
    walrus rejects tensor ops on it, so ACT+DVE split all of it.
  - KLD uses Ln(qy * V): the activation scale folds log(V) in, so one
    DVE multiply-accum gives sum qy*(log qy - log(1/V)); no separate
    sum(qy) pass.  Ln is the first activation so Square reuses its
    (natural_log) table -> exactly one ACT table load.
  - A burst of dummy matmuls at t=0 ramps the PE out of its low p-state
    before the real matmuls arrive.
  - DMA layout respects two measured HWDGE properties: the ring keeps
    only ~2 DMAs in flight, and each completion semaphore needs ~2us to
    collect its 16 per-engine increments -- so few chunks, each >=
    ~4KB/partition, a small lead chunk, and a small tail chunk; output
    partials store in two DMAs (early columns / tail columns).
"""

import contextlib
import ctypes
import os
import sys
import types

for _p in ("/opt/trn_rl_repo", "/root/.axon_site/_ro/trn_rl_repo"):
    if os.path.isdir(_p) and _p not in sys.path:
        sys.path.append(_p)

import numpy as np

# --- problem constants (hardcoded per spec) ---
B, S, N, D, V = 64, 128, 128, 512, 128
N_CORES = 8
BPC = B // N_CORES          # batches per core = 8
ALPHA, BETA, GAMMA, EPS = 1.0, 0.1, 1.0, 1e-20
MARK = (0, 29, 88, 117)

N_WARMUP = 36               # dummy matmuls to ramp the PE p-state
BAT_BYTES = 2048            # fp8 bytes per batch per partition

_CACHE = {}


def _install_ntff_hook_shim():
    """run_bass_kernel_spmd(trace=True) looks for antenv.axon_hooks, which
    this image lacks; recreate the ctypes hook against libaxon_pjrt.so."""
    if "antenv.axon_hooks" in sys.modules:
        return
    so_path = "/opt/axon/libaxon_pjrt.so"

    def _get_hook():
        if not os.path.exists(so_path):
            return None
        lib = ctypes.CDLL(so_path)
        if not hasattr(lib, "axon_start_nrt_profile"):
            return None
        lib.axon_start_nrt_profile.argtypes = [
            ctypes.POINTER(ctypes.c_int64), ctypes.c_size_t]
        lib.axon_start_nrt_profile.restype = ctypes.c_int64
        lib.axon_stop_nrt_profile.argtypes = [ctypes.c_char_p]
        lib.axon_stop_nrt_profile.restype = ctypes.c_int64

        @contextlib.contextmanager
        def _hook(output_dir, device_ids):
            import jax
            jax.devices()
            if device_ids:
                ids = (ctypes.c_int64 * len(device_ids))(*device_ids)
                rc = lib.axon_start_nrt_profile(ids, len(device_ids))
            else:
                rc = lib.axon_start_nrt_profile(None, 0)
            if rc != 0:
                raise RuntimeError(f"axon_start_nrt_profile rc={rc}")
            try:
                yield
            finally:
                n = lib.axon_stop_nrt_profile(str(output_dir).encode())
                if n < 0:
                    raise RuntimeError(f"axon_stop_nrt_profile rc={n}")

        return _hook

    mod = types.ModuleType("antenv.axon_hooks")
    mod.get_axon_ntff_profile_hook = _get_hook
    mod.set_axon_ntff_profile_hook = lambda h: None
    sys.modules["antenv.axon_hooks"] = mod


def _build_program():
    import concourse.bacc as bacc
    import concourse.tile as tile
    from concourse import mybir

    f32 = mybir.dt.float32
    bf16 = mybir.dt.bfloat16
    fp8 = mybir.dt.float8e4
    nc = bacc.Bacc(None, target_bir_lowering=False, debug=False)

    SQUARE = mybir.ActivationFunctionType.Square
    LN = mybir.ActivationFunctionType.Ln
    AL = mybir.AluOpType
    DR = mybir.MatmulPerfMode.DoubleRow

    # ---- per-core DRAM parameters (host pre-packed) ----
    # e2: one-hot gather pack, e[j, b*128+s] = (mapping[b,s] == j); slot 8
    # holds -I.  qb: [0:1024] qy (qb[v, b*128+s] = qy[b,s,v]), [1024:1280]
    # the "best" pack as bf16 bytes (cols [0:64] = A-halves, [64:128] =
    # B-halves, pre-scaled so one accum col == the epilogue term).
    d_e2 = nc.declare_dram_parameter("e2", [128, 1152], fp8, isOutput=False)
    d_qb = nc.declare_dram_parameter("qb", [128, 1280], fp8, isOutput=False)
    # per batch 2048B: [rz*4 | zs*4 | pts.w | masks/16 | pts_gt.w | masks_gt/16]
    d_bat = nc.declare_dram_parameter("bat", [128, BPC * BAT_BYTES], fp8, isOutput=False)
    d_out = nc.declare_dram_parameter("o_part", [128, 64], f32, isOutput=True)

    with tile.TileContext(nc) as tc:
        with contextlib.ExitStack() as ctx:
            sb = ctx.enter_context(tc.tile_pool(name="sb", bufs=1))
            psp = ctx.enter_context(tc.tile_pool(name="ps", bufs=1, space="PSUM"))

            # ---- tiles ----
            t_e2 = sb.tile([128, 1152], fp8)
            t_qb = sb.tile([128, 1280], fp8)
            t_bat = sb.tile([128, BPC * BAT_BYTES], fp8)
            t_ones = sb.tile([1, 128], bf16)
            t_lnq = sb.tile([128, BPC * V], bf16)
            t_jk = sb.tile([128, BPC * V], bf16)      # kld junk out
            t_ja = sb.tile([128, 2048], bf16)         # ACT square junk out
            t_ed = sb.tile([128, 64], f32)            # epilogue diffs
            t_ej = sb.tile([128, 64], f32)            # epilogue junk out
            a_out = sb.tile([128, 64], f32)
            ps = psp.tile([128, 4096], f32)           # all 8 PSUM banks

            e3 = t_e2[:].rearrange("p (a m) -> p a m", m=128)
            t_qy = t_qb[:, 0:1024]
            t_best = t_qb[:, 1024:1280].bitcast(bf16)     # [128, 128]

            # ---- PE warm-up: dummy matmuls ramp the p-state ----
            nc.vector.memset(t_ones[:], 1.0)
            for _ in range(N_WARMUP):
                nc.tensor.matmul(ps[:, 3072:3200], lhsT=t_ones[:],
                                 rhs=t_ones[:], start=True, stop=True)

            # ---- input DMAs (sync HWDGE ring, ordered by need) ----
            # The HWDGE ring keeps only ~2 DMAs in flight, and each DMA's
            # completion semaphore takes ~2us to collect its 16 per-engine
            # increments -- so use FEW chunks, each carrying >= ~2us of data
            # (>=4KB/partition), with a small lead chunk (b0) to start
            # compute early.
            def bat_dma(b_lo, b_hi):
                nc.sync.dma_start(
                    out=t_bat[:, b_lo * BAT_BYTES:b_hi * BAT_BYTES],
                    in_=d_bat.ap()[:, b_lo * BAT_BYTES:b_hi * BAT_BYTES])

            # header DMAs ride the ACT HWDGE ring (ACT is idle early) so
            # the sync ring starts streaming batch data immediately.
            nc.scalar.dma_start(out=t_e2[:], in_=d_e2.ap())
            nc.scalar.dma_start(out=t_qb[:], in_=d_qb.ap())
            bat_dma(0, 1)
            bat_dma(1, 3)
            bat_dma(3, 6)
            bat_dma(6, 8)

            # ---- KLD: Ln(qy*V) folds log(V); DVE does multiply+accum ----
            # fp8 qy is always >= 2^-10 (inputs are uniform(1e-3, 1)), so no
            # eps bias is needed under the log.  (Pool rejects
            # TensorScalarPtr; DVE dual-SBUF reads are fine — only dual-PSUM
            # reads are forbidden.)
            nc.scalar.activation(out=t_lnq[:], in_=t_qy[:], func=LN,
                                 scale=float(V), bias=0.0)
            nc.vector.scalar_tensor_tensor(
                out=t_jk[:], in0=t_qy[:], scalar=1.0, in1=t_lnq[:],
                op0=AL.bypass, op1=AL.mult, accum_out=a_out[:, 8:9])

            # ---- epilogue ("best" terms): DVE sub, DVE square-accum ----
            nc.vector.scalar_tensor_tensor(
                out=t_ed[:], in0=t_best[:, 0:64], scalar=0.0,
                in1=t_best[:, 64:128], op0=AL.bypass, op1=AL.subtract)
            nc.vector.scalar_tensor_tensor(
                out=t_ej[:], in0=t_ed[:], scalar=1.0, in1=t_ed[:],
                op0=AL.bypass, op1=AL.mult, accum_out=a_out[:, 9:10])

            # ---- fused gather-minus-gt matmuls (DoubleRow fp8) ----
            # batch b -> PSUM slot (b % 4) * 1024
            def mm_batch(b):
                lhsT = e3[:, b:9:8 - b, :]          # [128, 2, 128]: E_b, -I
                base = b * BAT_BYTES
                r = b % 4
                rhs1 = t_bat[:, base:base + 1024].rearrange(
                    "p (k c) -> p k c", k=2)
                rhs2 = t_bat[:, base + 1024:base + 2048].rearrange(
                    "p (k c) -> p k c", k=2)
                nc.tensor.matmul(ps[:, r * 1024:r * 1024 + 512], lhsT=lhsT,
                                 rhs=rhs1, start=True, stop=True, perf_mode=DR)
                nc.tensor.matmul(ps[:, r * 1024 + 512:r * 1024 + 1024],
                                 lhsT=lhsT, rhs=rhs2, start=True, stop=True,
                                 perf_mode=DR)

            def sq_act(lo_col, n_col, out_col):
                nc.scalar.activation(
                    out=t_ja[:, 0:n_col],
                    in_=ps[:, lo_col:lo_col + n_col], func=SQUARE,
                    accum_out=a_out[:, out_col:out_col + 1])

            def sq_dve(lo_col, n_col, out_col):
                # DVE may read only ONE operand from PSUM, so squares run as
                # bn_stats: per 512-block (count, mean, n*var) for even/odd
                # element sets; host reconstructs sum(x^2) = M2 + n*mean^2.
                for k in range(n_col // 512):
                    nc.vector.bn_stats(
                        out=a_out[:, out_col + 6 * k:out_col + 6 * k + 6],
                        in_=ps[:, lo_col + 512 * k:lo_col + 512 * (k + 1)])

            # a_out columns ordered by completion time so the early block
            # [0:28] can store while the tail [28:48] still computes.
            # b0: ACT single square (earliest possible engine start)
            mm_batch(0)
            sq_act(0, 1024, 0)
            # (b1,b2): ACT pair over the contiguous PSUM range [1024:3072]
            mm_batch(1); mm_batch(2)
            sq_act(1024, 2048, 1)
            # b3: DVE bn_stats
            mm_batch(3)
            sq_dve(3072, 1024, 16)
            # (b4,b5): ACT pair (slots 0,1 freed by sq0 / pair12)
            mm_batch(4); mm_batch(5)
            sq_act(0, 2048, 28)
            # b6 -> DVE in full (ACT is still busy with pair45 then);
            # b7 (last chunk) -> halves split across both engines
            mm_batch(6)
            sq_dve(2048, 1024, 32)
            mm_batch(7)
            sq_act(3072, 512, 30)
            sq_dve(3584, 512, 44)

            # ---- store partials: early block, then the tail block ----
            nc.sync.dma_start(out=d_out.ap()[:, 0:28], in_=a_out[:, 0:28])
            nc.sync.dma_start(out=d_out.ap()[:, 28:52], in_=a_out[:, 28:52])

    nc.compile()
    return nc


def _get_program():
    if "nc" not in _CACHE:
        _CACHE["nc"] = _build_program()
    return _CACHE["nc"]


def _shard_inputs(inputs):
    """Split the full B=64 inputs into 8 per-core input maps (fp8 packs)."""
    import ml_dtypes
    fp8 = ml_dtypes.float8_e4m3fn
    bf16 = ml_dtypes.bfloat16
    f = lambda k: np.asarray(inputs[k], dtype=np.float32)

    # pts columns: weight sqrt(2) on MARK'ed landmark points folds the
    # landmark sum into the disk sum (both /(B*S) in the reference).
    wpts = np.ones((N, 2), dtype=np.float32)
    wpts[list(MARK), :] = np.sqrt(np.float32(2.0))
    wpts_f = wpts.reshape(1, 1, N, 2)

    rz4 = f("rzs") * np.float32(4.0)                    # [B,S,D]
    zs4 = f("zs") * np.float32(4.0)
    ptsw = f("pts") * wpts_f                            # [B,S,N,2]
    ptsgw = f("pts_gt") * wpts_f
    msc = np.float32(1.0 / 16.0)
    mk = f("masks") * msc
    mkg = f("masks_gt") * msc

    # bat[j, b, 0:512]=rz4, [512:1024]=zs4, [1024:1280]=ptsw,
    # [1280:1536]=masks/16, [1536:1792]=pts_gt.w, [1792:2048]=masks_gt/16
    bat = np.empty((B, S, BAT_BYTES), dtype=np.float32)
    bat[:, :, 0:512] = rz4
    bat[:, :, 512:1024] = zs4
    bat[:, :, 1024:1280] = ptsw.reshape(B, S, 256)
    bat[:, :, 1280:1536] = mk.reshape(B, S, 256)
    bat[:, :, 1536:1792] = ptsgw.reshape(B, S, 256)
    bat[:, :, 1792:2048] = mkg.reshape(B, S, 256)
    bat8 = bat.astype(fp8)                              # [B, S, 2048]

    qy8 = f("qy").astype(fp8)                           # [B, S, V]

    # "best" pack scales: one accumulator column == the exact epilogue
    # loss contribution gamma*(best_reg + best_auto + alpha*best_seg).
    s_rz = np.float32(1.0 / np.sqrt(B * D))
    bn2 = B * N * 2
    s_pt = np.float32(1.0 / np.sqrt(float(bn2) * float(B * N)))
    w_pt = np.full((N, 2), s_pt, dtype=np.float32)
    w_pt[list(MARK), :] *= np.float32(np.sqrt(1.0 + bn2))
    w_mask = np.zeros((N, 2), dtype=np.float32)
    w_mask[32:96, :] = np.float32(1.0 / np.sqrt(B * 64 * 2))

    mapping = np.asarray(inputs["mapping"]).astype(np.int64)

    in_maps = []
    for c in range(N_CORES):
        lo, hi = c * BPC, (c + 1) * BPC
        m_core = mapping[lo:hi]                          # [8, 128]

        e = np.zeros((128, 9, 128), dtype=np.float32)
        bi = np.repeat(np.arange(BPC), S)
        si = np.tile(np.arange(S), BPC)
        e[m_core.reshape(-1), bi, si] = 1.0
        e[:, 8, :] = -np.eye(128, dtype=np.float32)

        # best pack halves [128, 64] each
        a_half = np.concatenate([
            (f("best_rz")[lo:hi] * s_rz).reshape(128, 32),
            (f("best_pt")[lo:hi] * w_pt).reshape(128, 16),
            (f("best_mask")[lo:hi] * w_mask).reshape(128, 16)], axis=1)
        b_half = np.concatenate([
            (f("logits")[lo:hi] * s_rz).reshape(128, 32),
            (f("best_pt_gt")[lo:hi] * w_pt).reshape(128, 16),
            (f("best_mask_gt")[lo:hi] * w_mask).reshape(128, 16)], axis=1)
        best = np.concatenate([a_half, b_half], axis=1).astype(bf16)

        qb = np.empty((128, 1280), dtype=fp8)
        qb[:, 0:1024] = qy8[lo:hi].transpose(2, 0, 1).reshape(128, BPC * V)
        qb[:, 1024:1280] = np.ascontiguousarray(best).view(np.uint8).view(fp8)

        m = {
            "e2": e.reshape(128, 9 * 128).astype(fp8),
            "qb": qb,
            "bat": np.ascontiguousarray(
                bat8[lo:hi].transpose(1, 0, 2).reshape(128, BPC * BAT_BYTES)),
        }
        in_maps.append(m)
    return in_maps


def _combine(results):
    """Host-side float64 reduction of the per-core partial sums."""

    def bn_sumsq(o, lo, blocks):
        # per 512-block bn_stats sextet: [cnt_e, mean_e, M2_e, cnt_o,
        # mean_o, M2_o]; sum(x^2) = M2 + cnt * mean^2 for each half.
        v = o[:, lo:lo + 6 * blocks].reshape(128, blocks, 2, 3)
        cnt, mean, m2 = v[..., 0], v[..., 1], v[..., 2]
        return (m2 + cnt * mean * mean).sum()

    s_main = s_kld = s_epi = 0.0
    for r in results:
        o = r["o_part"].astype(np.float64)
        s_main += (o[:, 0].sum() + o[:, 1].sum() + o[:, 28].sum()
                   + o[:, 30].sum())
        s_main += (bn_sumsq(o, 16, 2) + bn_sumsq(o, 32, 2)
                   + bn_sumsq(o, 44, 1))
        s_kld += o[:, 8].sum()
        s_epi += o[:, 9].sum()
    ret = s_main / (B * S) + BETA * s_kld / (B * S) + GAMMA * s_epi
    return np.float32(ret * B)


def run_sharded(inputs, trace=False):
    """Compile (cached), run on the 8 cores, return (scalar, BassKernelResults)."""
    _install_ntff_hook_shim()
    from concourse.bass_utils import run_bass_kernel_spmd

    nc = _get_program()
    in_maps = _shard_inputs(inputs)
    res = run_bass_kernel_spmd(nc, in_maps, list(range(N_CORES)), trace=trace)
    return _combine(res.results), res


def kernel(**inputs) -> np.ndarray:
    out, _ = run_sharded(inputs, trace=False)
    return out
